# revision 19
# baseline (speedup 1.0000x reference)
"""Trainium2 Bass kernel for CGL contrastive region loss.

Problem: proj (96, 256, 64, 64) f32 = 3 stacked views of B=32 images.
Only views 2 and 3 (aug1/aug2) are used. From each image, 25 regions
(5x5 grid of 2x2 windows at centres {10..50}) are extracted over all 256
channels -> region vectors of D = 256*2*2 = 1024. Per image pair the loss
needs the 50x50 Gram matrix of the stacked normalized region vectors
[u1; u2]: the loss reduces to, per row r of G, LSE over the full row
excluding only the main diagonal entry (=1/TAU exactly), minus the
positive logit pos_r = S[r, (r+25)%50]. Data-parallel over batch.

Mapping (4 pairs/core on 8 cores):
  host: gather 2x2 windows into a (128, pairs*8*50) array per core:
        partition = channel%128, free = (pair, chunk k=(cb,dy,dx), view, region)
  device, per pair b:
        G_b (50x50, PSUM) = sum_k U_bk^T U_bk   (8 matmuls, K=128, f32r 1-pass)
        d   = diag(G_b)  (ident mul + row-reduce, hidden under the gram phase)
  inv  = sqrt(10)*rsqrt(d) computed on ACT as exp(-0.5*ln(d/10)) — both Ln
        and Exp live in the single act-func set `natural_log_exp_and_others`,
        so ONE table load (hidden under the input DMAs) serves every
        activation in the kernel.
  colscale broadcast binv4 = ones^T @ diag(inv) (one PE matmul)
  gcol_b = G_b * binv4_b          (per-pair DVE mul)
  ACT fused: esum_b = rowsum(exp(inv_r * gcol_b + (-10)))
        (per-partition scale AP = row scale; bias -10 is a valid logsumexp
        shift since diag(S)=1/TAU=10; accum_out gives the row sum for free)
  lse  = ln(esum - 1)             (ACT Ln with bias -1 removes the diagonal
        exp(0)=1 term; equals true row-LSE minus 10)
  pos  = diag of the S12/S21 blocks, from gcol * inv (DVE, off-path)
  total = sum(lse) - 2*sum(pos) + 10*50*NB   (the +10 shift restored)
  output: per-core scalar total/(2*R*B_global) written via reg_load +
        reg_save (posted TENSOR_STORE) — no output DMA ring round-trip.

Span overheads trimmed: the Bass-init const memsets + entry all-engine
barrier are deleted from the BIR (the NRT preamble already provides two
all-engine rendezvous, and no const APs are referenced), letting the
input DMA triggers issue ~1.3us earlier. Tile tail uses the sem-only
drain barrier (FastTailTileContext). The NRT preamble (~5.6us) and
postamble semaphore wipe (~6.5us) are runtime-injected and immovable.
"""

import numpy as np

NB = 4                    # pairs per core
NCORES = 8
R = 25
FREE = NB * 8 * 50        # 1600 free elements per core
_CENTRES = (10, 20, 30, 40, 50)

# cf layout: (50, 453) f32
#   [0:200)   ident4: 4 horizontally tiled 50x50 identities
#   [200:400) mask4: -1e30 on each block diagonal, 0 elsewhere (added to G
#             during the PSUM->SBUF copy: exactly kills the main diagonal,
#             which the ln(esum-1) trick cannot do once the colscale matrix
#             passes through the f32r matmul quantization)
#   [400:450) ones 50x50 (f32r stationary for the colscale matmul;
#             col 400 doubles as the ones column for the final partition sum)
#   [450] -10.0   [451] 0.0
_CF_COLS = 452

_nc_cache = None


def _build_consts():
    ident = np.eye(50, dtype=np.float32)
    cf = np.zeros((50, _CF_COLS), dtype=np.float32)
    for b in range(4):
        cf[:, b * 50 : (b + 1) * 50] = ident
        cf[:, 200 + b * 50 : 200 + (b + 1) * 50] = ident * np.float32(-1e30)
    cf[:, 400:450] = 1.0
    cf[:, 450] = -10.0
    cf[:, 451] = 0.0
    return cf


def _strip_init_overhead(nc):
    """Remove the Bass-init const memsets and entry all-engine barrier from
    the 'main' block. No const APs are referenced by this kernel, and the
    NRT preamble already synchronizes all engines before the program runs."""
    from concourse import mybir

    for func in nc.m.functions:
        for blk in func.blocks:
            if blk.name != "main":
                continue
            kept = []
            for inst in blk.instructions:
                if isinstance(
                    inst,
                    (mybir.InstMemset, mybir.InstDrain, mybir.InstEventSemaphore),
                ):
                    continue
                kept.append(inst)
            blk.instructions[:] = kept


def _build_nc():
    import concourse.bacc as bacc
    import concourse.tile as tile
    from concourse import mybir
    from concourse.vector_clock import ScopedClock

    class FastTailTileContext(tile.TileContext):
        """Tile tail without the two full all-engine barriers.

        The sync-engine drain already waits on the global vector clock
        (every instruction's sem tick), so once it completes nothing is
        in flight; a sem-only EVSEM barrier then orders the gpsimd
        sem_clears after it."""

        def _drain_and_barrier(self, tick_clock, wait_clock):
            drain_inst = self.nc.sync.drain()
            wait_clock.add_sem_waits(
                drain_inst.ins, ScopedClock({None: tick_clock.global_clock})
            )
            self.nc.all_engine_barrier(sem_only=True)
            popped = self.nc._tile_sem_poison_stack.pop()
            assert popped is self._sem_poison
            self.nc.clear_and_free_semaphores(list(self.sems.allocated().values()))

    f32 = mybir.dt.float32
    f32r = mybir.dt.float32r
    Alu = mybir.AluOpType
    Act = mybir.ActivationFunctionType
    X = mybir.AxisListType.X

    nc = bacc.Bacc("TRN2", target_bir_lowering=False, debug=False)
    u_dram = nc.dram_tensor("u", [128, FREE], f32r, kind="ExternalInput").ap()
    # cf is f32r so the ones block can feed the f32r matmul; every const
    # value (0, 1, -10, -1) is exact in f32r, and DVE/ACT consumers use a
    # plain-f32 bitcast view.
    cf_dram = nc.dram_tensor("cf", [50, _CF_COLS], f32r, kind="ExternalInput").ap()
    out_dram = nc.dram_tensor("out", [1, 1], f32, kind="ExternalOutput").ap()

    def blk(ap, f=50):
        return ap.rearrange("p (b f) -> p b f", f=f)

    # raw (non-tile) SBUF scalar for the final result so the post-tile
    # reg_load sees a concrete (non-symbolic) access pattern
    res_t = nc.alloc_sbuf_tensor("res_scalar", [1, 1], f32)

    with FastTailTileContext(nc) as tc:
        with (
            tc.tile_pool(name="data", bufs=1) as data,
            tc.tile_pool(name="consts", bufs=1) as consts,
            tc.tile_pool(name="work", bufs=2) as work,
            tc.tile_pool(name="psg", bufs=4, space="PSUM") as psg,
            tc.tile_pool(name="psb", bufs=1, space="PSUM") as psb,
            tc.tile_pool(name="pst", bufs=1, space="PSUM") as pst,
        ):
            Q = FREE // 4
            ubs = []
            cf = consts.tile([50, _CF_COLS], f32r)
            for b in range(NB):
                ubq = data.tile([128, Q], f32r, tag=f"ub{b}")
                eng = nc.sync if b % 2 == 0 else nc.scalar
                eng.dma_start(ubq[:], u_dram[:, b * Q : (b + 1) * Q])
                ubs.append(ubq)
                if b == 1:
                    # consts ride the scalar HWDGE ring right after u1:
                    # small (50KB), lands before the first diag extraction
                    nc.scalar.dma_start(cf[:], cf_dram)

            cff = cf[:].bitcast(f32)
            ident4 = cff[:, 0:200]
            ident = cff[:, 0:50]
            mask4 = cff[:, 200:400]
            ones50r = cf[:, 400:450]
            ones_col = cff[:, 400:401]
            b_m10 = cff[:, 450:451]
            b_zero = cff[:, 451:452]

            # dummy Ln+Exp on a memset scratch (no DMA deps): pulls BOTH act
            # table loads (natural_log + exp_and_others -> the two table
            # slots) to the head of the ACT queue, fully hidden under the
            # input DMAs; the real activations then never reload a table
            tscr = work.tile([1, 1], f32, tag="tscr")
            nc.vector.memset(tscr[:], 1.0)
            nc.scalar.activation(tscr[:], tscr[:], Act.Ln, bias=tscr[:])
            nc.scalar.activation(tscr[:], tscr[:], Act.Exp, bias=tscr[:])

            gps = []
            gsb = work.tile([50, 200], f32, tag="gsb")
            for b in range(NB):
                gp = psg.tile([50, 50], f32, tag="g")
                for k in range(8):
                    sl = ubs[b][:, k * 50 : (k + 1) * 50]
                    nc.tensor.matmul(gp[:], sl, sl, start=(k == 0), stop=(k == 7))
                gps.append(gp)
                # PSUM -> SBUF move doubling as the diagonal mask add
                # (hidden under later gram chains); off-diagonal unchanged,
                # main diagonal driven to -1e30 so its exp is exactly 0
                nc.vector.tensor_add(
                    gsb[:, b * 50 : (b + 1) * 50], gp[:],
                    mask4[:, b * 50 : (b + 1) * 50],
                )

            # squared norms from block diagonals (hidden under the gram phase)
            dmul = work.tile([50, 200], f32, tag="dmul")
            dsq = work.tile([50, NB], f32, tag="dsq")
            for b in range(NB):
                nc.vector.tensor_mul(
                    dmul[:, b * 50 : (b + 1) * 50], gps[b][:], ident
                )
                nc.vector.reduce_sum(
                    dsq[:, b : b + 1],
                    dmul[:, b * 50 : (b + 1) * 50].unsqueeze(1),
                    axis=X,
                )

            # inv = sqrt(10)*rsqrt(d) = exp(-0.5*ln(d/10)) on ACT
            lnd = work.tile([50, NB], f32, tag="lnd")
            nc.scalar.activation(lnd[:], dsq[:], Act.Ln, bias=b_zero, scale=0.1)
            inv = work.tile([50, NB], f32, tag="inv")
            nc.scalar.activation(inv[:], lnd[:], Act.Exp, bias=b_zero, scale=-0.5)

            # colscale: binv4[p, (b,c)] = inv[c, b] via ones^T @ diag(inv)
            invrep = inv[:].unsqueeze(2).broadcast_to([50, NB, 50])
            dinv = work.tile([50, 200], f32r, tag="dinv")
            nc.vector.tensor_mul(blk(dinv[:]), blk(ident4), invrep)
            binv4 = psb.tile([50, 200], f32, tag="binv4")
            nc.tensor.matmul(binv4[:], ones50r, dinv[:], start=True, stop=True)

            # per pair: gcol_b = G_b * colscale; fused ACT does
            # esum_b = rowsum(exp(inv_r * gcol_b - 10))
            gcol = work.tile([50, 200], f32, tag="gcol")
            escr = work.tile([50, 50], f32, tag="escr")
            esum = work.tile([50, NB], f32, tag="esum")
            for b in range(NB):
                nc.vector.tensor_mul(
                    gcol[:, b * 50 : (b + 1) * 50],
                    gsb[:, b * 50 : (b + 1) * 50],
                    binv4[:, b * 50 : (b + 1) * 50],
                )
                nc.scalar.activation(
                    escr[:],
                    gcol[:, b * 50 : (b + 1) * 50],
                    Act.Exp,
                    bias=b_m10,
                    scale=inv[:, b : b + 1],
                    accum_out=esum[:, b : b + 1],
                )

            # positives: pos_r = gcol[r, 25+r] * inv_r  (S12/S21 diag), DVE
            pmul = work.tile([25, NB * 25], f32, tag="pmul")
            nc.vector.tensor_mul(
                blk(pmul[:], f=25),
                blk(gcol[0:25, :])[:, :, 25:50],
                blk(ident4[0:25])[:, :, 0:25],
            )
            posf = consts.tile([50, NB], f32)
            nc.vector.memset(posf[:], 0.0)
            nc.vector.reduce_sum(posf[0:25, :], blk(pmul[:], f=25), axis=X)
            posr = work.tile([50, NB], f32, tag="posr")
            nc.vector.tensor_mul(posr[:], posf[:], inv[:])

            # lse (shifted by -10); the diagonal is already exactly absent
            lnes = work.tile([50, NB], f32, tag="lnes")
            nc.scalar.activation(lnes[:], esum[:], Act.Ln, bias=b_zero)

            # total = sum(lnes) - 2*sum(posr) + 10*50*NB (shift restored)
            lsesum = work.tile([50, 1], f32, tag="lsesum")
            nc.vector.reduce_sum(lsesum[:], lnes[:], axis=X)
            possum = work.tile([50, 1], f32, tag="possum")
            nc.vector.reduce_sum(possum[:], posr[:], axis=X)
            acc = work.tile([50, 1], f32, tag="acc")
            nc.vector.scalar_tensor_tensor(
                acc[:], possum[:], -2.0, lsesum[:], op0=Alu.mult, op1=Alu.add
            )
            tp = pst.tile([1, 1], f32, tag="tot")
            nc.tensor.matmul(tp[:], acc[:], ones_col, start=True, stop=True)
            scale = 1.0 / (2.0 * R * NB * NCORES)
            nc.vector.tensor_scalar(
                res_t.ap(), tp[:], scale, (10.0 * 50 * NB) * scale,
                op0=Alu.mult, op1=Alu.add,
            )

    # posted TENSOR_STORE of the scalar result: no output DMA round-trip.
    # Runs after the tile drain barrier, before the NRT postamble.
    # (registers are untyped 32-bit — move the f32 bits via an i32 view)
    i32 = mybir.dt.int32
    with nc.gpsimd.register("resreg") as rreg:
        nc.gpsimd.reg_load(rreg, res_t.ap().bitcast(i32))
        nc.gpsimd.reg_save(out_dram.bitcast(i32), rreg)

    _strip_init_overhead(nc)
    nc.compile()
    return nc


def get_nc():
    global _nc_cache
    if _nc_cache is None:
        _nc_cache = _build_nc()
    return _nc_cache


def pack_inputs(proj: np.ndarray) -> np.ndarray:
    """(96,256,64,64) -> (128, 32, 8, 50): partition=c%128,
    free=(pair, chunk k=(cb,dy,dx), view, region rh*5+rw)."""
    win = np.array([[c - 1, c] for c in _CENTRES])  # (5, 2): rows/cols of each window
    v = np.stack([proj[32:64], proj[64:96]], axis=1)  # (32, 2, 256, 64, 64)
    g = v[:, :, :, win[:, :, None, None], win[None, None, :, :]]  # (32,2,256,5,2,5,2)
    g = g.reshape(32, 2, 2, 128, 5, 2, 5, 2)  # b, view, cb, c', rh, dy, rw, dx
    arr = np.transpose(g, (3, 0, 2, 5, 7, 1, 4, 6))  # c', b, cb, dy, dx, view, rh, rw
    return np.ascontiguousarray(arr).reshape(128, 32, 8, 50)


def kernel(proj: np.ndarray) -> np.ndarray:
    from concourse.bass_utils import run_bass_kernel_spmd

    nc = get_nc()
    arr = pack_inputs(np.asarray(proj))
    cf = _build_consts()
    in_maps = [
        {
            "u": np.ascontiguousarray(arr[:, c * NB : (c + 1) * NB]).reshape(128, FREE),
            "cf": cf,
        }
        for c in range(NCORES)
    ]
    results = run_bass_kernel_spmd(nc, in_maps, list(range(NCORES))).results
    total = 0.0
    for r in results:
        total += float(r["out"][0, 0])
    return np.float32(total)


# revision 20
# speedup vs baseline: 1.2133x; 1.2133x over previous
"""Trainium2 Bass kernel for CGL contrastive region loss.

Problem: proj (96, 256, 64, 64) f32 = 3 stacked views of B=32 images.
Only views 2 and 3 (aug1/aug2) are used. From each image, 25 regions
(5x5 grid of 2x2 windows at centres {10..50}) are extracted over all 256
channels -> region vectors of D = 256*2*2 = 1024. Per image pair the
loss reduces to: for each row r of the 50x50 Gram matrix G of the
stacked normalized regions [u1;u2] (scaled by 1/TAU), LSE over the full
row excluding only the main diagonal entry, minus the positive logit
pos_r = S[r, (r+25)%50]. Data-parallel over batch (4 pairs/core, 8
cores), scalar partials summed on the host.

Device pipeline per core (all 4 pairs batched in 50x200 tiles):
  bf16 inputs, 4 gram chains (8 matmuls each, K=128) into ONE 50x200
  PSUM tile.  diag: one ident4 mul + one blocked reduce -> d [50,4].
  inv = sqrt(10)*rsqrt(d) = exp(-0.5*ln(d/10)) on ACT.
  G+mask -> SBUF (mask = -1e30 on each block diagonal: exact row-LSE
  diagonal removal; doubles as the PSUM->SBUF move).
  colscale broadcast binv4 = ones^T @ diag(inv) (one f32r PE matmul),
  S = (G+mask) * binv4 * inv_row (two DVE muls), eall = exp(S - 10)
  (one ACT op, valid LSE shift since diag(S)=10), esum = blocked row
  reduce.  lse-10 = ln(esum) (ACT).  pos: -2*pos via a -2*I25 constant
  mul + reduce, summed together with ln(esum) in one 50x8 reduce.
  total = partition-sum matmul; scale + (+10 shift restored) constant
  folded into one tensor_scalar; result leaves via reg_load + posted
  TENSOR_STORE (no output DMA ring round-trip).

ACT tables: every activation (Ln, Exp) is served by the single function
set `natural_log_exp_and_others`, forced by pointing both bacc's
insert_act_table_loads and walrus (BASS_ACT_ROOT_JSON_PATH) at a
patched act_info.json in which no other set contains exp/ln. One table
load, pulled to the head of the ACT queue by a dummy activation and
hidden under the input DMAs. (The default greedy assignment alternates
natural_log/exp_and_others sets, reloading a 1.3us table at every
Ln<->Exp transition, several on the critical path.)

Span overheads trimmed: Bass-init const memsets + entry all-engine
barrier deleted from the BIR (the NRT preamble already runs two
all-engine rendezvous and no const APs are referenced), so the input
DMA triggers issue right after the NRT preamble. Tile tail uses a
sem-only drain barrier. The NRT preamble (~5.5-7us) and postamble
semaphore wipe (~6.5us) are runtime-injected and immovable.
"""

import os
import numpy as np

NB = 4                    # pairs per core
NCORES = 8
R = 25
FREE = NB * 8 * 50        # 1600 free elements per core
_CENTRES = (10, 20, 30, 40, 50)

# cf layout (f32r bits, mostly consumed through an f32 bitcast view):
#   [0:200)   ident4: 4 horizontally tiled 50x50 identities
#   [200:400) mask4: -1e30 on each 50-block diagonal, 0 elsewhere
#   [400:450) ones 50x50 (f32r stationary for the colscale matmul;
#             col 400 doubles as the ones column for the final sum)
#   [450:550) negident25_4: rows 0:25 = 4 blocks of -2 * I25 (positive-
#             logit extraction, the -2 loss weight pre-folded)
#   [550] -10.0   [551] 0.0
_CF_COLS = 552

_nc_cache = None


def _build_consts():
    ident = np.eye(50, dtype=np.float32)
    cf = np.zeros((50, _CF_COLS), dtype=np.float32)
    for b in range(4):
        cf[:, b * 50 : (b + 1) * 50] = ident
        cf[:, 200 + b * 50 : 200 + (b + 1) * 50] = ident * np.float32(-1e30)
        cf[0:25, 450 + b * 25 : 450 + (b + 1) * 25] = np.eye(25) * np.float32(-2.0)
    cf[:, 400:450] = 1.0
    cf[:, 550] = -10.0
    cf[:, 551] = 0.0
    return cf


def _patched_act_root():
    """Stage a copy of the neuronxcc pwp table dir whose act_info.json
    leaves `natural_log_exp_and_others` as the only set containing exp or
    ln, so every activation resolves to one table set (single load)."""
    import json
    import shutil
    import tempfile

    import neuronxcc

    src = os.path.join(os.path.dirname(neuronxcc.__file__), "pwp", "pwp_bin_trainium")
    dst = os.path.join(tempfile.gettempdir(), "pwp_nlexp_%d" % os.getuid())
    marker = os.path.join(dst, ".patched_ok")
    if not os.path.exists(marker):
        if os.path.exists(dst):
            shutil.rmtree(dst)
        shutil.copytree(src, dst)
        p = os.path.join(dst, "act_info.json")
        os.chmod(p, 0o644)
        with open(p) as f:
            d = json.load(f)
        for e in d["act_func_sets"]:
            if e["name"] != "natural_log_exp_and_others":
                e["act"].pop("exp", None)
                e["act"].pop("ln", None)
        with open(p, "w") as f:
            json.dump(d, f)
        with open(marker, "w") as f:
            f.write("ok")
    return os.path.join(dst, "act_info.json")


def _apply_act_surgery():
    import functools
    import json

    import concourse.bacc as baccmod

    act_json = _patched_act_root()
    os.environ["BASS_ACT_ROOT_JSON_PATH"] = act_json

    @functools.cache
    def patched_tables(arch):
        from concourse import mybir

        with open(act_json) as f:
            d = json.load(f)
        return {
            e["name"]: {
                mybir.ActivationFunctionType.from_pwp(v) for v in e["act"].keys()
            }
            for e in d["act_func_sets"]
        }

    baccmod.get_activation_tables = patched_tables


def _strip_init_overhead(nc):
    """Remove the Bass-init const memsets and entry all-engine barrier from
    the 'main' block. No const APs are referenced by this kernel, and the
    NRT preamble already synchronizes all engines before the program runs."""
    from concourse import mybir

    for func in nc.m.functions:
        for blk in func.blocks:
            if blk.name != "main":
                continue
            kept = []
            for inst in blk.instructions:
                if isinstance(
                    inst,
                    (mybir.InstMemset, mybir.InstDrain, mybir.InstEventSemaphore),
                ):
                    continue
                kept.append(inst)
            blk.instructions[:] = kept


def _build_nc():
    _apply_act_surgery()

    import concourse.bacc as bacc
    import concourse.tile as tile
    from concourse import mybir
    from concourse.vector_clock import ScopedClock

    class FastTailTileContext(tile.TileContext):
        """Tile tail without the two full all-engine barriers.

        The sync-engine drain already waits on the global vector clock
        (every instruction's sem tick), so once it completes nothing is
        in flight; a sem-only EVSEM barrier then orders the gpsimd
        sem_clears after it."""

        def _drain_and_barrier(self, tick_clock, wait_clock):
            drain_inst = self.nc.sync.drain()
            wait_clock.add_sem_waits(
                drain_inst.ins, ScopedClock({None: tick_clock.global_clock})
            )
            self.nc.all_engine_barrier(sem_only=True)
            popped = self.nc._tile_sem_poison_stack.pop()
            assert popped is self._sem_poison
            self.nc.clear_and_free_semaphores(list(self.sems.allocated().values()))

    f32 = mybir.dt.float32
    f32r = mybir.dt.float32r
    bf16 = mybir.dt.bfloat16
    i32 = mybir.dt.int32
    Alu = mybir.AluOpType
    Act = mybir.ActivationFunctionType
    X = mybir.AxisListType.X

    nc = bacc.Bacc("TRN2", target_bir_lowering=False, debug=False)
    u_dram = nc.dram_tensor("u", [128, FREE], bf16, kind="ExternalInput").ap()
    cf_dram = nc.dram_tensor("cf", [50, _CF_COLS], f32r, kind="ExternalInput").ap()
    out_dram = nc.dram_tensor("out", [1, 1], f32, kind="ExternalOutput").ap()

    def blk(ap, f=50):
        return ap.rearrange("p (b f) -> p b f", f=f)

    # raw (non-tile) SBUF scalar for the final result so the post-tile
    # reg_load sees a concrete (non-symbolic) access pattern
    res_t = nc.alloc_sbuf_tensor("res_scalar", [1, 1], f32)

    with FastTailTileContext(nc) as tc:
        with (
            tc.tile_pool(name="data", bufs=1) as data,
            tc.tile_pool(name="consts", bufs=1) as consts,
            tc.tile_pool(name="work", bufs=2) as work,
            tc.tile_pool(name="psg", bufs=1, space="PSUM") as psg,
            tc.tile_pool(name="psb", bufs=1, space="PSUM") as psb,
            tc.tile_pool(name="pst", bufs=1, space="PSUM") as pst,
        ):
            Q = FREE // 4
            # u quarters: pairs 0,2 on the sync HWDGE ring, 1,3 on scalar;
            # the const tensor follows the last quarter on the sync ring
            # (needed later than any u data)
            ubs = []
            cf = consts.tile([50, _CF_COLS], f32r)
            for b in range(NB):
                ubq = data.tile([128, Q], bf16, tag=f"ub{b}")
                eng = nc.sync if b % 2 == 0 else nc.scalar
                eng.dma_start(ubq[:], u_dram[:, b * Q : (b + 1) * Q])
                ubs.append(ubq)
            nc.sync.dma_start(cf[:], cf_dram)

            cff = cf[:].bitcast(f32)
            ident4 = cff[:, 0:200]
            mask4 = cff[:, 200:400]
            ones50r = cf[:, 400:450]
            ones_col = cff[:, 400:401]
            negident = cff[:, 450:550]
            b_m10 = cff[:, 550:551]
            b_zero = cff[:, 551:552]

            # dummy activation on a memset scratch (no DMA deps): pulls the
            # single ACT table load to the head of the ACT queue, fully
            # hidden under the input DMAs
            tscr = work.tile([1, 1], f32, tag="tscr")
            nc.vector.memset(tscr[:], 1.0)
            nc.scalar.activation(tscr[:], tscr[:], Act.Exp, bias=tscr[:])

            # zero rows 25:50 of the -2*pos half of the final-sum tile
            fin = work.tile([50, 2 * NB], f32, tag="fin")
            nc.vector.memset(fin[:], 0.0)

            # 4 gram chains into one 50x200 PSUM tile
            gpall = psg.tile([50, 200], f32, tag="g")
            for b in range(NB):
                gslice = gpall[:, b * 50 : (b + 1) * 50]
                for k in range(8):
                    sl = ubs[b][:, k * 50 : (k + 1) * 50]
                    nc.tensor.matmul(gslice, sl, sl, start=(k == 0), stop=(k == 7))

            # block diagonals -> squared norms d [50,4]
            dmul = work.tile([50, 200], f32, tag="dmul")
            nc.vector.tensor_mul(dmul[:], gpall[:], ident4)
            dsq = work.tile([50, NB], f32, tag="dsq")
            nc.vector.reduce_sum(dsq[:], blk(dmul[:]), axis=X)

            # inv = sqrt(10)*rsqrt(d) = exp(-0.5*ln(d/10)) on ACT
            lnd = work.tile([50, NB], f32, tag="lnd")
            nc.scalar.activation(lnd[:], dsq[:], Act.Ln, bias=b_zero, scale=0.1)
            inv = work.tile([50, NB], f32, tag="inv")
            nc.scalar.activation(inv[:], lnd[:], Act.Exp, bias=b_zero, scale=-0.5)

            # PSUM -> SBUF move doubling as the exact diagonal kill
            gsb = work.tile([50, 200], f32, tag="gsb")
            nc.vector.tensor_add(gsb[:], gpall[:], mask4)

            # colscale row: binv4[p,(b,c)] = inv[c,b] via ones^T @ diag(inv)
            invrep = inv[:].unsqueeze(2).broadcast_to([50, NB, 50])
            dinv = work.tile([50, 200], f32r, tag="dinv")
            nc.vector.tensor_mul(blk(dinv[:]), blk(ident4), invrep)
            binv4 = psb.tile([50, 200], f32, tag="binv4")
            nc.tensor.matmul(binv4[:], ones50r, dinv[:], start=True, stop=True)

            # S = (G+mask) * colscale * rowscale; eall = exp(S-10)
            gcol = work.tile([50, 200], f32, tag="gcol")
            nc.vector.tensor_mul(gcol[:], gsb[:], binv4[:])
            rowsc = work.tile([50, 200], f32, tag="rowsc")
            nc.vector.tensor_mul(blk(rowsc[:]), blk(gcol[:]), invrep)
            eall = work.tile([50, 200], f32, tag="eall")
            nc.scalar.activation(eall[:], rowsc[:], Act.Exp, bias=b_m10)
            esum = work.tile([50, NB], f32, tag="esum")
            nc.vector.reduce_sum(esum[:], blk(eall[:]), axis=X)

            # -2 * positives from the S12/S21 block diagonals of S
            pmul = work.tile([25, NB * 25], f32, tag="pmul")
            nc.vector.tensor_mul(
                blk(pmul[:], f=25),
                blk(rowsc[0:25, :])[:, :, 25:50],
                blk(negident[0:25], f=25),
            )
            nc.vector.reduce_sum(fin[0:25, NB : 2 * NB], blk(pmul[:], f=25), axis=X)

            # lse-10 = ln(esum); one 50x8 reduce sums lse and -2*pos rows
            nc.scalar.activation(fin[:, 0:NB], esum[:], Act.Ln, bias=b_zero)
            acc = work.tile([50, 1], f32, tag="acc")
            nc.vector.reduce_sum(acc[:], fin[:], axis=X)

            # partition sum -> scalar; restore the +10 LSE shift (10*50*NB)
            # and apply 1/(2*R*B) in the same op
            tp = pst.tile([1, 1], f32, tag="tot")
            nc.tensor.matmul(tp[:], acc[:], ones_col, start=True, stop=True)
            scale = 1.0 / (2.0 * R * NB * NCORES)
            nc.vector.tensor_scalar(
                res_t.ap(), tp[:], scale, (10.0 * 50 * NB) * scale,
                op0=Alu.mult, op1=Alu.add,
            )

    # posted TENSOR_STORE of the scalar result: no output DMA round-trip.
    # Runs after the tile drain barrier, before the NRT postamble.
    # (registers are untyped 32-bit — move the f32 bits via an i32 view)
    with nc.gpsimd.register("resreg") as rreg:
        nc.gpsimd.reg_load(rreg, res_t.ap().bitcast(i32))
        nc.gpsimd.reg_save(out_dram.bitcast(i32), rreg)

    _strip_init_overhead(nc)
    nc.compile()
    return nc


def get_nc():
    global _nc_cache
    if _nc_cache is None:
        _nc_cache = _build_nc()
    return _nc_cache


def pack_inputs(proj: np.ndarray) -> np.ndarray:
    """(96,256,64,64) -> (128, 32, 8, 50) bf16: partition=c%128,
    free=(pair, chunk k=(cb,dy,dx), view, region rh*5+rw)."""
    import ml_dtypes

    win = np.array([[c - 1, c] for c in _CENTRES])  # (5, 2): rows/cols of each window
    v = np.stack([proj[32:64], proj[64:96]], axis=1)  # (32, 2, 256, 64, 64)
    g = v[:, :, :, win[:, :, None, None], win[None, None, :, :]]  # (32,2,256,5,2,5,2)
    g = g.reshape(32, 2, 2, 128, 5, 2, 5, 2)  # b, view, cb, c', rh, dy, rw, dx
    arr = np.transpose(g, (3, 0, 2, 5, 7, 1, 4, 6))  # c', b, cb, dy, dx, view, rh, rw
    return np.ascontiguousarray(arr).reshape(128, 32, 8, 50).astype(ml_dtypes.bfloat16)


def kernel(proj: np.ndarray) -> np.ndarray:
    from concourse.bass_utils import run_bass_kernel_spmd

    nc = get_nc()
    arr = pack_inputs(np.asarray(proj))
    cf = _build_consts()
    in_maps = [
        {
            "u": np.ascontiguousarray(arr[:, c * NB : (c + 1) * NB]).reshape(128, FREE),
            "cf": cf,
        }
        for c in range(NCORES)
    ]
    results = run_bass_kernel_spmd(nc, in_maps, list(range(NCORES))).results
    total = 0.0
    for r in results:
        total += float(r["out"][0, 0])
    return np.float32(total)


# revision 25
# speedup vs baseline: 1.2471x; 1.0279x over previous
"""Trainium2 Bass kernel for CGL contrastive region loss.

Problem: proj (96, 256, 64, 64) f32 = 3 stacked views of B=32 images.
Only views 2 and 3 (aug1/aug2) are used. From each image, 25 regions
(5x5 grid of 2x2 windows at centres {10..50}) are extracted over all 256
channels -> region vectors of D = 256*2*2 = 1024. Per image pair the
loss reduces to: for each row r of the 50x50 Gram matrix G of the
stacked normalized regions [u1;u2] (scaled by 1/TAU), LSE over the full
row excluding only the main diagonal entry, minus the positive logit
pos_r = S[r, (r+25)%50]. Data-parallel over batch (4 pairs/core, 8
cores), scalar partials summed on the host.

Device pipeline per core (all 4 pairs batched in 50x200 tiles):
  bf16 inputs, 4 gram chains (8 matmuls each, K=128) into ONE 50x200
  PSUM tile.  diag: one ident4 mul + one blocked reduce -> d [50,4].
  inv = sqrt(10)*rsqrt(d) = exp(-0.5*ln(d/10)) on ACT.
  G+mask -> SBUF (mask = -1e30 on each block diagonal: exact row-LSE
  diagonal removal; doubles as the PSUM->SBUF move).
  colscale broadcast binv4 = ones^T @ diag(inv) (one f32r PE matmul),
  S = (G+mask) * binv4 * inv_row (two DVE muls), eall = exp(S - 10)
  (one ACT op, valid LSE shift since diag(S)=10), esum = blocked row
  reduce.  lse-10 = ln(esum) (ACT).  pos: -2*pos via a -2*I25 constant
  mul + reduce, summed together with ln(esum) in one 50x8 reduce.
  total = partition-sum matmul; scale + (+10 shift restored) constant
  folded into one tensor_scalar; result leaves via reg_load + posted
  TENSOR_STORE (no output DMA ring round-trip).

ACT tables: every activation (Ln, Exp) is served by the single function
set `natural_log_exp_and_others`, forced by pointing both bacc's
insert_act_table_loads and walrus (BASS_ACT_ROOT_JSON_PATH) at a
patched act_info.json in which no other set contains exp/ln. One table
load, pulled to the head of the ACT queue by a dummy activation and
hidden under the input DMAs. (The default greedy assignment alternates
natural_log/exp_and_others sets, reloading a 1.3us table at every
Ln<->Exp transition, several on the critical path.)

Span overheads trimmed: Bass-init const memsets + entry all-engine
barrier deleted from the BIR (the NRT preamble already runs two
all-engine rendezvous and no const APs are referenced), so the input
DMA triggers issue right after the NRT preamble. Tile tail uses a
sem-only drain barrier. The NRT preamble (~5.5-7us) and postamble
semaphore wipe (~6.5us) are runtime-injected and immovable.
"""

import os
import numpy as np

NB = 4                    # pairs per core
NCORES = 8
R = 25
FREE = NB * 8 * 50        # 1600 free elements per core
_CENTRES = (10, 20, 30, 40, 50)

# cf layout (f32r bits, mostly consumed through an f32 bitcast view;
# per-pair block constants are free-dim stride-0 broadcasts of one copy):
#   [0:50)    ident: 50x50 identity
#   [50:100)  mask: -1e30 on the diagonal, 0 elsewhere
#   [100:150) ones 50x50 (f32r stationary for the colscale matmul;
#             col 100 doubles as the ones column for the final sum)
#   [150:175) negident25: rows 0:25 = -2 * I25 (positive-logit
#             extraction, the -2 loss weight pre-folded)
#   [175] -10.0   [176] 0.0
_CF_COLS = 177

_nc_cache = None


def _build_consts():
    cf = np.zeros((50, _CF_COLS), dtype=np.float32)
    cf[:, 0:50] = np.eye(50)
    cf[:, 50:100] = np.eye(50) * np.float32(-1e30)
    cf[:, 100:150] = 1.0
    cf[0:25, 150:175] = np.eye(25) * np.float32(-2.0)
    cf[:, 175] = -10.0
    cf[:, 176] = 0.0
    return cf


def _patched_act_root():
    """Stage a copy of the neuronxcc pwp table dir whose act_info.json
    leaves `natural_log_exp_and_others` as the only set containing exp or
    ln, so every activation resolves to one table set (single load)."""
    import json
    import shutil
    import tempfile

    import neuronxcc

    src = os.path.join(os.path.dirname(neuronxcc.__file__), "pwp", "pwp_bin_trainium")
    dst = os.path.join(tempfile.gettempdir(), "pwp_nlexp_%d" % os.getuid())
    marker = os.path.join(dst, ".patched_ok")
    if not os.path.exists(marker):
        if os.path.exists(dst):
            shutil.rmtree(dst)
        shutil.copytree(src, dst)
        p = os.path.join(dst, "act_info.json")
        os.chmod(p, 0o644)
        with open(p) as f:
            d = json.load(f)
        for e in d["act_func_sets"]:
            if e["name"] != "natural_log_exp_and_others":
                e["act"].pop("exp", None)
                e["act"].pop("ln", None)
        with open(p, "w") as f:
            json.dump(d, f)
        with open(marker, "w") as f:
            f.write("ok")
    return os.path.join(dst, "act_info.json")


def _apply_act_surgery():
    import functools
    import json

    import concourse.bacc as baccmod

    act_json = _patched_act_root()
    os.environ["BASS_ACT_ROOT_JSON_PATH"] = act_json

    @functools.cache
    def patched_tables(arch):
        from concourse import mybir

        with open(act_json) as f:
            d = json.load(f)
        return {
            e["name"]: {
                mybir.ActivationFunctionType.from_pwp(v) for v in e["act"].keys()
            }
            for e in d["act_func_sets"]
        }

    baccmod.get_activation_tables = patched_tables


def _strip_init_overhead(nc):
    """Remove the Bass-init const memsets and entry all-engine barrier from
    the 'main' block. No const APs are referenced by this kernel, and the
    NRT preamble already synchronizes all engines before the program runs."""
    from concourse import mybir

    for func in nc.m.functions:
        for blk in func.blocks:
            if blk.name != "main":
                continue
            kept = []
            for inst in blk.instructions:
                if isinstance(
                    inst,
                    (mybir.InstMemset, mybir.InstDrain, mybir.InstEventSemaphore),
                ):
                    continue
                kept.append(inst)
            blk.instructions[:] = kept


def _build_nc():
    _apply_act_surgery()

    import concourse.bacc as bacc
    import concourse.tile as tile
    from concourse import mybir
    from concourse.vector_clock import ScopedClock

    class FastTailTileContext(tile.TileContext):
        """Tile tail without the two full all-engine barriers.

        The sync-engine drain already waits on the global vector clock
        (every instruction's sem tick), so once it completes nothing is
        in flight; a sem-only EVSEM barrier then orders the gpsimd
        sem_clears after it."""

        def _drain_and_barrier(self, tick_clock, wait_clock):
            drain_inst = self.nc.sync.drain()
            wait_clock.add_sem_waits(
                drain_inst.ins, ScopedClock({None: tick_clock.global_clock})
            )
            self.nc.all_engine_barrier(sem_only=True)
            popped = self.nc._tile_sem_poison_stack.pop()
            assert popped is self._sem_poison
            self.nc.clear_and_free_semaphores(list(self.sems.allocated().values()))

    f32 = mybir.dt.float32
    f32r = mybir.dt.float32r
    bf16 = mybir.dt.bfloat16
    i32 = mybir.dt.int32
    Alu = mybir.AluOpType
    Act = mybir.ActivationFunctionType
    X = mybir.AxisListType.X

    nc = bacc.Bacc("TRN2", target_bir_lowering=False, debug=False)
    u_dram = nc.dram_tensor("u", [128, FREE], bf16, kind="ExternalInput").ap()
    cf_dram = nc.dram_tensor("cf", [50, _CF_COLS], f32r, kind="ExternalInput").ap()
    out_dram = nc.dram_tensor("out", [1, 1], f32, kind="ExternalOutput").ap()

    def blk(ap, f=50):
        return ap.rearrange("p (b f) -> p b f", f=f)

    # raw (non-tile) SBUF scalar for the final result so the post-tile
    # reg_load sees a concrete (non-symbolic) access pattern
    res_t = nc.alloc_sbuf_tensor("res_scalar", [1, 1], f32)

    with FastTailTileContext(nc) as tc:
        with (
            tc.tile_pool(name="data", bufs=1) as data,
            tc.tile_pool(name="consts", bufs=1) as consts,
            tc.tile_pool(name="work", bufs=2) as work,
            tc.tile_pool(name="psg", bufs=1, space="PSUM") as psg,
            tc.tile_pool(name="psb", bufs=1, space="PSUM") as psb,
            tc.tile_pool(name="pst", bufs=1, space="PSUM") as pst,
        ):
            H = FREE // 2
            # u halves (1600B rows keep the DMA rings at full burst
            # efficiency): pairs 0-1 on the sync HWDGE ring, 2-3 on
            # scalar; the small const tensor follows on the sync ring
            ubs = []
            cf = consts.tile([50, _CF_COLS], f32r)
            for h in range(2):
                ubh = data.tile([128, H], bf16, tag=f"ub{h}")
                eng = nc.sync if h == 0 else nc.scalar
                eng.dma_start(ubh[:], u_dram[:, h * H : (h + 1) * H])
                ubs.append(ubh)
            nc.sync.dma_start(cf[:], cf_dram)

            cff = cf[:].bitcast(f32)
            ident = cff[:, 0:50]
            mask = cff[:, 50:100]
            ones50r = cf[:, 100:150]
            ones_col = cff[:, 100:101]
            negident = cff[0:25, 150:175]
            b_m10 = cff[:, 175:176]
            b_zero = cff[:, 176:177]
            identB = ident.unsqueeze(1).broadcast_to([50, NB, 50])
            maskB = mask.unsqueeze(1).broadcast_to([50, NB, 50])
            negidentB = negident.unsqueeze(1).broadcast_to([25, NB, 25])

            # dummy activation on a memset scratch (no DMA deps): pulls the
            # single ACT table load to the head of the ACT queue, fully
            # hidden under the input DMAs
            tscr = work.tile([1, 1], f32, tag="tscr")
            nc.vector.memset(tscr[:], 1.0)
            nc.scalar.activation(tscr[:], tscr[:], Act.Exp, bias=tscr[:])

            # zero rows 25:50 of the -2*pos half of the final-sum tile
            fin = work.tile([50, 2 * NB], f32, tag="fin")
            nc.vector.memset(fin[:], 0.0)

            # 4 gram chains into one 50x200 PSUM tile
            gpall = psg.tile([50, 200], f32, tag="g")
            for b in range(NB):
                gslice = gpall[:, b * 50 : (b + 1) * 50]
                base = (b % 2) * 400
                for k in range(8):
                    sl = ubs[b // 2][:, base + k * 50 : base + (k + 1) * 50]
                    nc.tensor.matmul(gslice, sl, sl, start=(k == 0), stop=(k == 7))

            # block diagonals -> squared norms d [50,4]
            dmul = work.tile([50, 200], f32, tag="dmul")
            nc.vector.tensor_mul(blk(dmul[:]), blk(gpall[:]), identB)
            dsq = work.tile([50, NB], f32, tag="dsq")
            nc.vector.reduce_sum(dsq[:], blk(dmul[:]), axis=X)

            # inv = sqrt(10)*rsqrt(d) = exp(-0.5*ln(d/10)) on ACT
            lnd = work.tile([50, NB], f32, tag="lnd")
            nc.scalar.activation(lnd[:], dsq[:], Act.Ln, bias=b_zero, scale=0.1)
            inv = work.tile([50, NB], f32, tag="inv")
            nc.scalar.activation(inv[:], lnd[:], Act.Exp, bias=b_zero, scale=-0.5)

            # PSUM -> SBUF move doubling as the exact diagonal kill
            gsb = work.tile([50, 200], f32, tag="gsb")
            nc.vector.tensor_add(blk(gsb[:]), blk(gpall[:]), maskB)

            # colscale row: binv4[p,(b,c)] = inv[c,b] via ones^T @ diag(inv)
            invrep = inv[:].unsqueeze(2).broadcast_to([50, NB, 50])
            dinv = work.tile([50, 200], f32r, tag="dinv")
            nc.vector.tensor_mul(blk(dinv[:]), identB, invrep)
            binv4 = psb.tile([50, 200], f32, tag="binv4")
            nc.tensor.matmul(binv4[:], ones50r, dinv[:], start=True, stop=True)

            # S = (G+mask) * colscale * rowscale; eall = exp(S-10)
            gcol = work.tile([50, 200], f32, tag="gcol")
            nc.vector.tensor_mul(gcol[:], gsb[:], binv4[:])
            rowsc = work.tile([50, 200], f32, tag="rowsc")
            nc.vector.tensor_mul(blk(rowsc[:]), blk(gcol[:]), invrep)
            eall = work.tile([50, 200], f32, tag="eall")
            nc.scalar.activation(eall[:], rowsc[:], Act.Exp, bias=b_m10)
            esum = work.tile([50, NB], f32, tag="esum")
            nc.vector.reduce_sum(esum[:], blk(eall[:]), axis=X)

            # -2 * positives from the S12/S21 block diagonals of S
            pmul = work.tile([25, NB * 25], f32, tag="pmul")
            nc.vector.tensor_mul(
                blk(pmul[:], f=25),
                blk(rowsc[0:25, :])[:, :, 25:50],
                negidentB,
            )
            nc.vector.reduce_sum(fin[0:25, NB : 2 * NB], blk(pmul[:], f=25), axis=X)

            # lse-10 = ln(esum); one 50x8 reduce sums lse and -2*pos rows
            nc.scalar.activation(fin[:, 0:NB], esum[:], Act.Ln, bias=b_zero)
            acc = work.tile([50, 1], f32, tag="acc")
            nc.vector.reduce_sum(acc[:], fin[:], axis=X)

            # partition sum -> scalar; restore the +10 LSE shift (10*50*NB)
            # and apply 1/(2*R*B) in the same op
            tp = pst.tile([1, 1], f32, tag="tot")
            nc.tensor.matmul(tp[:], acc[:], ones_col, start=True, stop=True)
            scale = 1.0 / (2.0 * R * NB * NCORES)
            nc.vector.tensor_scalar(
                res_t.ap(), tp[:], scale, (10.0 * 50 * NB) * scale,
                op0=Alu.mult, op1=Alu.add,
            )

    # posted TENSOR_STORE of the scalar result: no output DMA round-trip.
    # Runs after the tile drain barrier, before the NRT postamble.
    # (registers are untyped 32-bit — move the f32 bits via an i32 view)
    with nc.gpsimd.register("resreg") as rreg:
        nc.gpsimd.reg_load(rreg, res_t.ap().bitcast(i32))
        nc.gpsimd.reg_save(out_dram.bitcast(i32), rreg)

    _strip_init_overhead(nc)
    nc.compile()
    return nc


def get_nc():
    global _nc_cache
    if _nc_cache is None:
        _nc_cache = _build_nc()
    return _nc_cache


def pack_inputs(proj: np.ndarray) -> np.ndarray:
    """(96,256,64,64) -> (128, 32, 8, 50) bf16: partition=c%128,
    free=(pair, chunk k=(cb,dy,dx), view, region rh*5+rw)."""
    import ml_dtypes

    win = np.array([[c - 1, c] for c in _CENTRES])  # (5, 2): rows/cols of each window
    v = np.stack([proj[32:64], proj[64:96]], axis=1)  # (32, 2, 256, 64, 64)
    g = v[:, :, :, win[:, :, None, None], win[None, None, :, :]]  # (32,2,256,5,2,5,2)
    g = g.reshape(32, 2, 2, 128, 5, 2, 5, 2)  # b, view, cb, c', rh, dy, rw, dx
    arr = np.transpose(g, (3, 0, 2, 5, 7, 1, 4, 6))  # c', b, cb, dy, dx, view, rh, rw
    return np.ascontiguousarray(arr).reshape(128, 32, 8, 50).astype(ml_dtypes.bfloat16)


def kernel(proj: np.ndarray) -> np.ndarray:
    from concourse.bass_utils import run_bass_kernel_spmd

    nc = get_nc()
    arr = pack_inputs(np.asarray(proj))
    cf = _build_consts()
    in_maps = [
        {
            "u": np.ascontiguousarray(arr[:, c * NB : (c + 1) * NB]).reshape(128, FREE),
            "cf": cf,
        }
        for c in range(NCORES)
    ]
    results = run_bass_kernel_spmd(nc, in_maps, list(range(NCORES))).results
    total = 0.0
    for r in results:
        total += float(r["out"][0, 0])
    return np.float32(total)
